# revision 3
# baseline (speedup 1.0000x reference)
"""DomainEncoder MoE kernel for Trainium2 (8 NeuronCores, expert-parallel).

Reference computes, for each of 32768 tokens, one of 8 expert MLPs
(Linear 256->1024, LayerNorm, ReLU, Linear 1024->256) selected by
domain_types, by running ALL experts on ALL tokens and masking (8x waste).

Strategy here: host-side dispatch (stable argsort by expert), one expert per
NeuronCore. Core d receives the tokens of expert d, padded to a common
capacity C, pre-transposed to [256, C] so features live on SBUF partitions
(the matmul contraction dim). The device program is a dense
MLP in "hT layout" (hidden dim on partitions), which makes both matmuls
transpose-free:

  MM1:  hT[hid,t]  = W1[din,hid].T-tiles @ xT[din,t]   (W1 is already lhsT)
  LN :  stats over hid = partition-direction sums via ones-vector matmuls,
        per-token mu/rstd broadcast back across partitions via K=1 matmuls
  MM2:  yT[out,t]  = W2[hid,out].T-tiles @ relu(norm(hT))

Host gathers yT back through the same permutation. Compute per core is
~C/4096 of the per-expert work instead of 8x, i.e. ~7.5x less than the
reference's static dispatch.
"""

import os
from contextlib import ExitStack

import numpy as np

import concourse.bass as bass
import concourse.tile as tile
from concourse import mybir
from concourse.bass_utils import run_bass_kernel_spmd

N_EXPERTS = 8
D_IN = 256
D_HID = 1024
D_OUT = 256
LN_EPS = 1e-5
TOK = 512  # token tile width (PSUM fp32 bank limit = 512 floats)
N_CORES = 8

# Matmul input dtype: "f32" (bit-accurate-ish, 2 PE passes) or "bf16" (2x PE
# throughput, ~1e-3 relative error).
_DT = os.environ.get("KERNEL_MM_DTYPE", "f32")

_F32 = mybir.dt.float32
_AF = mybir.ActivationFunctionType


def _mm_dt():
    return {
        "f32": mybir.dt.float32,
        "bf16": mybir.dt.bfloat16,
        "f32r": mybir.dt.float32r,
    }[_DT]


def _np_dt():
    if _DT == "bf16":
        import ml_dtypes

        return ml_dtypes.bfloat16
    return np.float32


def _split_sync_waits(nc, max_waits: int = 1):
    """Walrus's per-instruction sync-wait slots are scarce (Drain and Matmult
    both reject >~2). Hoist excess waits from any instruction onto
    EventSemaphore carriers inserted just before it on the same engine —
    per-engine program order makes that semantically identical."""
    n = 0
    for fn in nc.m.functions:
        for bb in fn.blocks:
            insts = list(bb.instructions)
            out = []
            changed = False
            for inst in insts:
                si = inst.sync_info
                waits = list(si.on_wait) if si and si.on_wait else []
                if len(waits) > max_waits:
                    for w in waits[:-max_waits]:
                        carrier = mybir.InstEventSemaphore(
                            name=f"W-split-{n}", ins=[], outs=[]
                        )
                        n += 1
                        carrier.engine = inst.engine
                        carrier.sync_info = mybir.SyncInfo(
                            on_wait=[w], on_update=[]
                        )
                        out.append(carrier)
                    inst.sync_info = mybir.SyncInfo(
                        on_wait=waits[-max_waits:],
                        on_update=list(si.on_update or []),
                    )
                    changed = True
                out.append(inst)
            if changed:
                bb.instructions = out


_BUILD_CACHE = {}


def _build(C: int):
    """Trace the single-core Bass program for capacity C (SPMD across 8)."""
    key = (C, _DT)
    if key in _BUILD_CACHE:
        return _BUILD_CACHE[key]

    dt = _mm_dt()
    nc = bass.Bass("TRN2", target_bir_lowering=False, debug=False)
    xT = nc.dram_tensor("xT", [D_IN, C], dt, kind="ExternalInput").ap()
    w1 = nc.dram_tensor("w1", [D_IN, D_HID], dt, kind="ExternalInput").ap()
    b1 = nc.dram_tensor("b1", [D_HID], _F32, kind="ExternalInput").ap()
    gamma = nc.dram_tensor("gamma", [D_HID], _F32, kind="ExternalInput").ap()
    beta = nc.dram_tensor("beta", [D_HID], _F32, kind="ExternalInput").ap()
    w2 = nc.dram_tensor("w2", [D_HID, D_OUT], dt, kind="ExternalInput").ap()
    b2 = nc.dram_tensor("b2", [D_OUT], _F32, kind="ExternalInput").ap()
    yT = nc.dram_tensor("yT", [D_OUT, C], _F32, kind="ExternalOutput").ap()

    nt = C // TOK
    KC = D_IN // 128  # 2 contraction chunks for MM1
    MH = D_HID // 128  # 8 hidden chunks
    MO = D_OUT // 128  # 2 output chunks
    inv_hid = 1.0 / D_HID

    with tile.TileContext(nc) as tc, ExitStack() as ctx:
        const = ctx.enter_context(tc.tile_pool(name="const", bufs=1))
        xp = ctx.enter_context(tc.tile_pool(name="xp", bufs=3))
        hpool = ctx.enter_context(tc.tile_pool(name="hpool", bufs=2))
        tpool = ctx.enter_context(tc.tile_pool(name="tpool", bufs=3))
        spool = ctx.enter_context(tc.tile_pool(name="spool", bufs=2))
        ypool = ctx.enter_context(tc.tile_pool(name="ypool", bufs=2))
        hp_ps = ctx.enter_context(tc.tile_pool(name="hp_ps", bufs=2, space="PSUM"))
        st_ps = ctx.enter_context(tc.tile_pool(name="st_ps", bufs=1, space="PSUM"))
        rep_ps = ctx.enter_context(tc.tile_pool(name="rep_ps", bufs=1, space="PSUM"))
        y_ps = ctx.enter_context(tc.tile_pool(name="y_ps", bufs=2, space="PSUM"))

        w1_sb = const.tile([128, KC, D_HID], dt)
        nc.sync.dma_start(out=w1_sb, in_=w1.rearrange("(k p) h -> p k h", p=128))
        w2_sb = const.tile([128, MH, D_OUT], dt)
        nc.sync.dma_start(out=w2_sb, in_=w2.rearrange("(k p) o -> p k o", p=128))
        b1_sb = const.tile([128, MH], _F32)
        nc.sync.dma_start(out=b1_sb, in_=b1.rearrange("(c p) -> p c", p=128))
        gamma_sb = const.tile([128, MH], _F32)
        nc.sync.dma_start(out=gamma_sb, in_=gamma.rearrange("(c p) -> p c", p=128))
        beta_sb = const.tile([128, MH], _F32)
        nc.sync.dma_start(out=beta_sb, in_=beta.rearrange("(c p) -> p c", p=128))
        b2_sb = const.tile([128, MO], _F32)
        nc.sync.dma_start(out=b2_sb, in_=b2.rearrange("(j p) -> p j", p=128))
        ones_col = const.tile([128, 1], dt)
        nc.vector.memset(ones_col, 1.0)
        ones_row = const.tile([1, 128], _F32)
        nc.vector.memset(ones_row, 1.0)
        eps_sb = const.tile([1, 1], _F32)
        nc.vector.memset(eps_sb, LN_EPS)

        for it in range(nt):
            sl = slice(it * TOK, (it + 1) * TOK)
            xt = xp.tile([128, KC, TOK], dt, tag="xt")
            nc.sync.dma_start(
                out=xt, in_=xT[:, sl].rearrange("(k p) t -> p k t", p=128)
            )

            h_sb = hpool.tile([128, MH, TOK], dt, tag="h")
            h2_sb = hpool.tile([128, MH, TOK], dt, tag="h2")
            hn_sb = hpool.tile([128, MH, TOK], dt, tag="hn")

            # MM1: hT chunks; bias applied on the PSUM->SBUF copy.
            for m in range(MH):
                hp = hp_ps.tile([128, TOK], _F32, tag="hp")
                for k in range(KC):
                    nc.tensor.matmul(
                        hp,
                        lhsT=w1_sb[:, k, m * 128 : (m + 1) * 128],
                        rhs=xt[:, k, :],
                        start=(k == 0),
                        stop=(k == KC - 1),
                    )
                nc.scalar.activation(
                    out=h_sb[:, m, :], in_=hp, func=_AF.Identity,
                    bias=b1_sb[:, m : m + 1],
                )
                nc.scalar.activation(
                    out=h2_sb[:, m, :], in_=hp, func=_AF.Square,
                    bias=b1_sb[:, m : m + 1],
                )

            # Partition-direction sums over hid via ones-vector matmuls.
            musum = st_ps.tile([1, TOK], _F32, tag="musum")
            for c in range(MH):
                nc.tensor.matmul(
                    musum, lhsT=ones_col, rhs=h_sb[:, c, :],
                    start=(c == 0), stop=(c == MH - 1),
                )
            sqsum = st_ps.tile([1, TOK], _F32, tag="sqsum")
            for c in range(MH):
                nc.tensor.matmul(
                    sqsum, lhsT=ones_col, rhs=h2_sb[:, c, :],
                    start=(c == 0), stop=(c == MH - 1),
                )

            # Finalize per-token stats ([1, TOK], cheap).
            mu = spool.tile([1, TOK], _F32, tag="mu")
            nc.scalar.mul(mu, musum, inv_hid)
            negmu = spool.tile([1, TOK], _F32, tag="negmu")
            nc.scalar.mul(negmu, musum, -inv_hid)
            ex2 = spool.tile([1, TOK], _F32, tag="ex2")
            nc.scalar.mul(ex2, sqsum, inv_hid)
            mu2 = spool.tile([1, TOK], _F32, tag="mu2")
            nc.vector.tensor_mul(mu2, mu, mu)
            var = spool.tile([1, TOK], _F32, tag="var")
            nc.vector.tensor_sub(var, ex2, mu2)
            sd = spool.tile([1, TOK], _F32, tag="sd")
            nc.scalar.activation(out=sd, in_=var, func=_AF.Sqrt, bias=eps_sb)
            rstd = spool.tile([1, TOK], _F32, tag="rstd")
            nc.vector.reciprocal(rstd, sd)
            bvec = spool.tile([1, TOK], _F32, tag="bvec")
            nc.vector.tensor_mul(bvec, negmu, rstd)

            # Broadcast per-token scalars across partitions: rank-1 matmuls.
            arep = rep_ps.tile([128, TOK], _F32, tag="arep")
            nc.tensor.matmul(arep, lhsT=ones_row, rhs=rstd, start=True, stop=True)
            brep = rep_ps.tile([128, TOK], _F32, tag="brep")
            nc.tensor.matmul(brep, lhsT=ones_row, rhs=bvec, start=True, stop=True)

            # Normalize + affine + ReLU per chunk; hn = Relu(gamma*t + beta).
            for c in range(MH):
                t1 = tpool.tile([128, TOK], _F32, tag="t1")
                nc.vector.tensor_mul(t1, h_sb[:, c, :], arep)
                t2 = tpool.tile([128, TOK], _F32, tag="t2")
                nc.vector.tensor_add(t2, t1, brep)
                nc.scalar.activation(
                    out=hn_sb[:, c, :], in_=t2, func=_AF.Relu,
                    bias=beta_sb[:, c : c + 1], scale=gamma_sb[:, c : c + 1],
                )

            # MM2: yT chunks.
            y_sb = ypool.tile([128, MO, TOK], _F32, tag="y")
            for j in range(MO):
                yp = y_ps.tile([128, TOK], _F32, tag="yp")
                for k in range(MH):
                    nc.tensor.matmul(
                        yp,
                        lhsT=w2_sb[:, k, j * 128 : (j + 1) * 128],
                        rhs=hn_sb[:, k, :],
                        start=(k == 0),
                        stop=(k == MH - 1),
                    )
                nc.scalar.activation(
                    out=y_sb[:, j, :], in_=yp, func=_AF.Identity,
                    bias=b2_sb[:, j : j + 1],
                )
            nc.sync.dma_start(
                out=yT[:, sl].rearrange("(j p) t -> p j t", p=128), in_=y_sb
            )

    _split_sync_waits(nc)
    _BUILD_CACHE[key] = nc
    return nc


def _prepare(inputs):
    """Host-side dispatch: sort tokens by expert, pad, transpose."""
    x = np.asarray(inputs["x"], dtype=np.float32)
    dom = np.asarray(inputs["domain_types"]).astype(np.int64)
    W1 = np.asarray(inputs["W1"], dtype=np.float32)
    b1 = np.asarray(inputs["b1"], dtype=np.float32)
    gamma = np.asarray(inputs["gamma"], dtype=np.float32)
    beta = np.asarray(inputs["beta"], dtype=np.float32)
    W2 = np.asarray(inputs["W2"], dtype=np.float32)
    b2 = np.asarray(inputs["b2"], dtype=np.float32)

    n = x.shape[0]
    order = np.argsort(dom, kind="stable")
    counts = np.bincount(dom, minlength=N_EXPERTS)
    maxc = int(counts.max())
    C = max(TOK, -(-maxc // TOK) * TOK)

    np_dt = _np_dt()
    in_maps = []
    idx_list = []
    off = 0
    for d in range(N_EXPERTS):
        nd = int(counts[d])
        idx = order[off : off + nd]
        off += nd
        idx_list.append(idx)
        xTd = np.zeros((D_IN, C), dtype=np_dt)
        xTd[:, :nd] = x[idx].T.astype(np_dt, copy=False)
        in_maps.append(
            {
                "xT": xTd,
                "w1": W1[d].astype(np_dt, copy=False),
                "b1": b1[d],
                "gamma": gamma[d],
                "beta": beta[d],
                "w2": W2[d].astype(np_dt, copy=False),
                "b2": b2[d],
            }
        )
    meta = {"n": n, "C": C, "idx_list": idx_list, "out_dtype": x.dtype}
    return in_maps, meta


def _finish(results, meta):
    out = np.zeros((meta["n"], D_OUT), dtype=meta["out_dtype"])
    for d in range(N_EXPERTS):
        idx = meta["idx_list"][d]
        if len(idx):
            out[idx] = results[d]["yT"][:, : len(idx)].T
    return out


def kernel(**inputs) -> np.ndarray:
    in_maps, meta = _prepare(inputs)
    nc = _build(meta["C"])
    res = run_bass_kernel_spmd(nc, in_maps, core_ids=list(range(N_CORES)))
    return _finish(res.results, meta)


# revision 6
# speedup vs baseline: 2.7166x; 2.7166x over previous
"""DomainEncoder MoE kernel for Trainium2 (8 NeuronCores, expert-parallel).

Reference computes, for each of 32768 tokens, one of 8 expert MLPs
(Linear 256->1024, LayerNorm, ReLU, Linear 1024->256) selected by
domain_types, by running ALL experts on ALL tokens and masking (8x waste).

Strategy here: host-side dispatch (stable argsort by expert), one expert per
NeuronCore. Core d receives the tokens of expert d, padded to a common
capacity C, pre-transposed to [256, C] so features live on SBUF partitions
(the matmul contraction dim). The device program is a dense
MLP in "hT layout" (hidden dim on partitions), which makes both matmuls
transpose-free:

  MM1:  hT[hid,t]  = W1[din,hid].T-tiles @ xT[din,t]   (W1 is already lhsT)
  LN :  stats over hid = partition-direction sums via ones-vector matmuls,
        per-token mu/rstd broadcast back across partitions via K=1 matmuls
  MM2:  yT[out,t]  = W2[hid,out].T-tiles @ relu(norm(hT))

Host gathers yT back through the same permutation. Compute per core is
~C/4096 of the per-expert work instead of 8x, i.e. ~7.5x less than the
reference's static dispatch.
"""

import os
from contextlib import ExitStack

import numpy as np

import concourse.bass as bass
import concourse.tile as tile
from concourse import mybir
from concourse.bass_utils import run_bass_kernel_spmd

N_EXPERTS = 8
D_IN = 256
D_HID = 1024
D_OUT = 256
LN_EPS = 1e-5
TOK = 512  # token tile width (PSUM fp32 bank limit = 512 floats)
N_CORES = 8

# Matmul input dtype: "f32" (bit-accurate-ish, 2 PE passes) or "bf16" (2x PE
# throughput, ~1e-3 relative error).
_DT = os.environ.get("KERNEL_MM_DTYPE", "f32")

_F32 = mybir.dt.float32
_AF = mybir.ActivationFunctionType


def _mm_dt():
    return {
        "f32": mybir.dt.float32,
        "bf16": mybir.dt.bfloat16,
        "f32r": mybir.dt.float32r,
    }[_DT]


def _np_dt():
    if _DT == "bf16":
        import ml_dtypes

        return ml_dtypes.bfloat16
    return np.float32


def _split_sync_waits(nc, max_waits: int = 1):
    """Walrus's per-instruction sync-wait slots are scarce (Drain and Matmult
    both reject >~2). Hoist excess waits from any instruction onto
    EventSemaphore carriers inserted just before it on the same engine —
    per-engine program order makes that semantically identical."""
    n = 0
    for fn in nc.m.functions:
        for bb in fn.blocks:
            insts = list(bb.instructions)
            out = []
            changed = False
            for inst in insts:
                si = inst.sync_info
                waits = list(si.on_wait) if si and si.on_wait else []
                if len(waits) > max_waits:
                    for w in waits[:-max_waits]:
                        carrier = mybir.InstEventSemaphore(
                            name=f"W-split-{n}", ins=[], outs=[]
                        )
                        n += 1
                        carrier.engine = inst.engine
                        carrier.sync_info = mybir.SyncInfo(
                            on_wait=[w], on_update=[]
                        )
                        out.append(carrier)
                    inst.sync_info = mybir.SyncInfo(
                        on_wait=waits[-max_waits:],
                        on_update=list(si.on_update or []),
                    )
                    changed = True
                out.append(inst)
            if changed:
                bb.instructions = out


_BUILD_CACHE = {}


def _build(C: int):
    """Trace the single-core Bass program for capacity C (SPMD across 8)."""
    key = (C, _DT)
    if key in _BUILD_CACHE:
        return _BUILD_CACHE[key]

    dt = _mm_dt()
    nc = bass.Bass("TRN2", target_bir_lowering=False, debug=False)
    xT = nc.dram_tensor("xT", [D_IN, C], dt, kind="ExternalInput").ap()
    w1 = nc.dram_tensor("w1", [D_IN, D_HID], dt, kind="ExternalInput").ap()
    b1 = nc.dram_tensor("b1", [D_HID], _F32, kind="ExternalInput").ap()
    gamma = nc.dram_tensor("gamma", [D_HID], _F32, kind="ExternalInput").ap()
    beta = nc.dram_tensor("beta", [D_HID], _F32, kind="ExternalInput").ap()
    w2 = nc.dram_tensor("w2", [D_HID, D_OUT], dt, kind="ExternalInput").ap()
    b2 = nc.dram_tensor("b2", [D_OUT], _F32, kind="ExternalInput").ap()
    yT = nc.dram_tensor("yT", [D_OUT, C], _F32, kind="ExternalOutput").ap()

    nt = C // TOK
    KC = D_IN // 128  # 2 contraction chunks for MM1
    MH = D_HID // 128  # 8 hidden chunks
    MO = D_OUT // 128  # 2 output chunks
    inv_hid = 1.0 / D_HID

    with tile.TileContext(nc) as tc, ExitStack() as ctx:
        const = ctx.enter_context(tc.tile_pool(name="const", bufs=1))
        xp = ctx.enter_context(tc.tile_pool(name="xp", bufs=3))
        hpool = ctx.enter_context(tc.tile_pool(name="hpool", bufs=2))
        tpool = ctx.enter_context(tc.tile_pool(name="tpool", bufs=3))
        spool = ctx.enter_context(tc.tile_pool(name="spool", bufs=2))
        ypool = ctx.enter_context(tc.tile_pool(name="ypool", bufs=2))
        hp_ps = ctx.enter_context(tc.tile_pool(name="hp_ps", bufs=2, space="PSUM"))
        st_ps = ctx.enter_context(tc.tile_pool(name="st_ps", bufs=1, space="PSUM"))
        rep_ps = ctx.enter_context(tc.tile_pool(name="rep_ps", bufs=1, space="PSUM"))
        y_ps = ctx.enter_context(tc.tile_pool(name="y_ps", bufs=2, space="PSUM"))

        w1_sb = const.tile([128, KC, D_HID], dt)
        nc.sync.dma_start(out=w1_sb, in_=w1.rearrange("(k p) h -> p k h", p=128))
        w2_sb = const.tile([128, MH, D_OUT], dt)
        nc.sync.dma_start(out=w2_sb, in_=w2.rearrange("(k p) o -> p k o", p=128))
        b1_sb = const.tile([128, MH], _F32)
        nc.sync.dma_start(out=b1_sb, in_=b1.rearrange("(c p) -> p c", p=128))
        gamma_sb = const.tile([128, MH], _F32)
        nc.sync.dma_start(out=gamma_sb, in_=gamma.rearrange("(c p) -> p c", p=128))
        beta_sb = const.tile([128, MH], _F32)
        nc.sync.dma_start(out=beta_sb, in_=beta.rearrange("(c p) -> p c", p=128))
        b2_sb = const.tile([128, MO], _F32)
        nc.sync.dma_start(out=b2_sb, in_=b2.rearrange("(j p) -> p j", p=128))
        ones_col = const.tile([128, 1], dt)
        nc.vector.memset(ones_col, 1.0)
        ones_row = const.tile([1, 128], _F32)
        nc.vector.memset(ones_row, 1.0)
        eps_sb = const.tile([1, 1], _F32)
        nc.vector.memset(eps_sb, LN_EPS)

        for it in range(nt):
            sl = slice(it * TOK, (it + 1) * TOK)
            xt = xp.tile([128, KC, TOK], dt, tag="xt")
            nc.sync.dma_start(
                out=xt, in_=xT[:, sl].rearrange("(k p) t -> p k t", p=128)
            )

            h_sb = hpool.tile([128, MH, TOK], dt, tag="h")
            h2_sb = hpool.tile([128, MH, TOK], dt, tag="h2")
            hn_sb = hpool.tile([128, MH, TOK], dt, tag="hn")

            # MM1: hT chunks; bias applied on the PSUM->SBUF copy.
            for m in range(MH):
                hp = hp_ps.tile([128, TOK], _F32, tag="hp")
                for k in range(KC):
                    nc.tensor.matmul(
                        hp,
                        lhsT=w1_sb[:, k, m * 128 : (m + 1) * 128],
                        rhs=xt[:, k, :],
                        start=(k == 0),
                        stop=(k == KC - 1),
                    )
                nc.scalar.activation(
                    out=h_sb[:, m, :], in_=hp, func=_AF.Identity,
                    bias=b1_sb[:, m : m + 1],
                )
                nc.vector.tensor_mul(
                    h2_sb[:, m, :], h_sb[:, m, :], h_sb[:, m, :]
                )

            # Partition-direction sums over hid via ones-vector matmuls.
            musum = st_ps.tile([1, TOK], _F32, tag="musum")
            for c in range(MH):
                nc.tensor.matmul(
                    musum, lhsT=ones_col, rhs=h_sb[:, c, :],
                    start=(c == 0), stop=(c == MH - 1),
                )
            sqsum = st_ps.tile([1, TOK], _F32, tag="sqsum")
            for c in range(MH):
                nc.tensor.matmul(
                    sqsum, lhsT=ones_col, rhs=h2_sb[:, c, :],
                    start=(c == 0), stop=(c == MH - 1),
                )

            # Finalize per-token stats ([1, TOK], cheap).
            mu = spool.tile([1, TOK], _F32, tag="mu")
            nc.scalar.mul(mu, musum, inv_hid)
            negmu = spool.tile([1, TOK], _F32, tag="negmu")
            nc.scalar.mul(negmu, musum, -inv_hid)
            ex2 = spool.tile([1, TOK], _F32, tag="ex2")
            nc.scalar.mul(ex2, sqsum, inv_hid)
            mu2 = spool.tile([1, TOK], _F32, tag="mu2")
            nc.vector.tensor_mul(mu2, mu, mu)
            var = spool.tile([1, TOK], _F32, tag="var")
            nc.vector.tensor_sub(var, ex2, mu2)
            # rstd = 1/sqrt(var+eps) = exp(-0.5*ln(var+eps)); DVE reciprocal
            # is ~6.5ns/elem on one lane, ACT Ln/Exp are far cheaper here.
            lnv = spool.tile([1, TOK], _F32, tag="lnv")
            nc.scalar.activation(out=lnv, in_=var, func=_AF.Ln, bias=eps_sb)
            rstd = spool.tile([1, TOK], _F32, tag="rstd")
            nc.scalar.activation(out=rstd, in_=lnv, func=_AF.Exp, scale=-0.5)
            bvec = spool.tile([1, TOK], _F32, tag="bvec")
            nc.vector.tensor_mul(bvec, negmu, rstd)

            # Broadcast per-token scalars across partitions: rank-1 matmuls.
            arep = rep_ps.tile([128, TOK], _F32, tag="arep")
            nc.tensor.matmul(arep, lhsT=ones_row, rhs=rstd, start=True, stop=True)
            brep = rep_ps.tile([128, TOK], _F32, tag="brep")
            nc.tensor.matmul(brep, lhsT=ones_row, rhs=bvec, start=True, stop=True)

            # Normalize + affine + ReLU per chunk; hn = Relu(gamma*t + beta).
            for c in range(MH):
                t1 = tpool.tile([128, TOK], _F32, tag="t1")
                nc.vector.tensor_mul(t1, h_sb[:, c, :], arep)
                t2 = tpool.tile([128, TOK], _F32, tag="t2")
                nc.vector.tensor_add(t2, t1, brep)
                nc.scalar.activation(
                    out=hn_sb[:, c, :], in_=t2, func=_AF.Relu,
                    bias=beta_sb[:, c : c + 1], scale=gamma_sb[:, c : c + 1],
                )

            # MM2: yT chunks.
            y_sb = ypool.tile([128, MO, TOK], _F32, tag="y")
            for j in range(MO):
                yp = y_ps.tile([128, TOK], _F32, tag="yp")
                for k in range(MH):
                    nc.tensor.matmul(
                        yp,
                        lhsT=w2_sb[:, k, j * 128 : (j + 1) * 128],
                        rhs=hn_sb[:, k, :],
                        start=(k == 0),
                        stop=(k == MH - 1),
                    )
                nc.scalar.activation(
                    out=y_sb[:, j, :], in_=yp, func=_AF.Identity,
                    bias=b2_sb[:, j : j + 1],
                )
            nc.sync.dma_start(
                out=yT[:, sl].rearrange("(j p) t -> p j t", p=128), in_=y_sb
            )

    _split_sync_waits(nc)
    _BUILD_CACHE[key] = nc
    return nc


def _prepare(inputs):
    """Host-side dispatch: sort tokens by expert, pad, transpose."""
    x = np.asarray(inputs["x"], dtype=np.float32)
    dom = np.asarray(inputs["domain_types"]).astype(np.int64)
    W1 = np.asarray(inputs["W1"], dtype=np.float32)
    b1 = np.asarray(inputs["b1"], dtype=np.float32)
    gamma = np.asarray(inputs["gamma"], dtype=np.float32)
    beta = np.asarray(inputs["beta"], dtype=np.float32)
    W2 = np.asarray(inputs["W2"], dtype=np.float32)
    b2 = np.asarray(inputs["b2"], dtype=np.float32)

    n = x.shape[0]
    order = np.argsort(dom, kind="stable")
    counts = np.bincount(dom, minlength=N_EXPERTS)
    maxc = int(counts.max())
    C = max(TOK, -(-maxc // TOK) * TOK)

    np_dt = _np_dt()
    in_maps = []
    idx_list = []
    off = 0
    for d in range(N_EXPERTS):
        nd = int(counts[d])
        idx = order[off : off + nd]
        off += nd
        idx_list.append(idx)
        xTd = np.zeros((D_IN, C), dtype=np_dt)
        xTd[:, :nd] = x[idx].T.astype(np_dt, copy=False)
        in_maps.append(
            {
                "xT": xTd,
                "w1": W1[d].astype(np_dt, copy=False),
                "b1": b1[d],
                "gamma": gamma[d],
                "beta": beta[d],
                "w2": W2[d].astype(np_dt, copy=False),
                "b2": b2[d],
            }
        )
    meta = {"n": n, "C": C, "idx_list": idx_list, "out_dtype": x.dtype}
    return in_maps, meta


def _finish(results, meta):
    out = np.zeros((meta["n"], D_OUT), dtype=meta["out_dtype"])
    for d in range(N_EXPERTS):
        idx = meta["idx_list"][d]
        if len(idx):
            out[idx] = results[d]["yT"][:, : len(idx)].T
    return out


def kernel(**inputs) -> np.ndarray:
    in_maps, meta = _prepare(inputs)
    nc = _build(meta["C"])
    res = run_bass_kernel_spmd(nc, in_maps, core_ids=list(range(N_CORES)))
    return _finish(res.results, meta)


# revision 10
# speedup vs baseline: 3.2265x; 1.1877x over previous
"""DomainEncoder MoE kernel for Trainium2 (8 NeuronCores, expert-parallel).

Reference computes, for each of 32768 tokens, one of 8 expert MLPs
(Linear 256->1024, LayerNorm, ReLU, Linear 1024->256) selected by
domain_types, by running ALL experts on ALL tokens and masking (8x waste).

Strategy: host-side dispatch (stable argsort by expert), one expert per
NeuronCore. Core d receives the tokens of expert d, padded to a common
capacity C, pre-transposed to [256, C] so features live on SBUF partitions
(the matmul contraction dim). The device program is a dense MLP in
"hT layout" (hidden dim on partitions), making both matmuls transpose-free:

  MM1:  hT[hid,t]  = W1[din,hid].T-tiles @ xT[din,t]   (W1 is already lhsT)
  LN :  mean folded into MM1 via host-precomputed W1.mean(1); E[h^2] via
        ones-vector matmuls over h^2; per-token mu/rstd broadcast back
        across partitions via rank-1 (K=1) matmuls
  MM2:  yT[out,t]  = W2[hid,out].T-tiles @ relu(gamma*norm(hT)+beta)

Host gathers yT back through the same permutation. Compute per core is
~C/4096 of one expert's work instead of 8x all-expert work.
"""

import os
from contextlib import ExitStack

import numpy as np

import concourse.bass as bass
import concourse.tile as tile
from concourse import mybir
from concourse.bass_utils import run_bass_kernel_spmd

N_EXPERTS = 8
D_IN = 256
D_HID = 1024
D_OUT = 256
LN_EPS = 1e-5
TOK = 512  # max token tile width (PSUM fp32 bank limit = 512 floats)
N_CORES = 8

# Matmul input dtype: "f32" (bit-accurate, 2 PE passes each) or "bf16".
_DT = os.environ.get("KERNEL_MM_DTYPE", "bf16")

_F32 = mybir.dt.float32
_F16 = mybir.dt.float16
_AF = mybir.ActivationFunctionType


def _mm_dt():
    return {
        "f32": mybir.dt.float32,
        "bf16": mybir.dt.bfloat16,
        "f32r": mybir.dt.float32r,
    }[_DT]


def _np_dt():
    if _DT == "bf16":
        import ml_dtypes

        return ml_dtypes.bfloat16
    return np.float32


def _split_sync_waits(nc, max_waits: int = 1):
    """Walrus's per-instruction sync-wait slots are scarce. Hoist excess
    waits from any instruction onto EventSemaphore carriers inserted just
    before it on the same engine — per-engine program order makes that
    semantically identical."""
    n = 0
    for fn in nc.m.functions:
        for bb in fn.blocks:
            insts = list(bb.instructions)
            out = []
            changed = False
            for inst in insts:
                si = inst.sync_info
                waits = list(si.on_wait) if si and si.on_wait else []
                limit = max_waits
                if type(inst).__name__ == "InstDrain":
                    limit = 1
                if len(waits) > limit:
                    for w in waits[:-limit]:
                        carrier = mybir.InstEventSemaphore(
                            name=f"W-split-{n}", ins=[], outs=[]
                        )
                        n += 1
                        carrier.engine = inst.engine
                        carrier.sync_info = mybir.SyncInfo(
                            on_wait=[w], on_update=[]
                        )
                        out.append(carrier)
                    inst.sync_info = mybir.SyncInfo(
                        on_wait=waits[-limit:],
                        on_update=list(si.on_update or []),
                    )
                    changed = True
                out.append(inst)
            if changed:
                bb.instructions = out


def _bcast2(ap):
    """View a [128, W] AP as [128, 2, W] with a stride-0 middle dim."""
    return bass.AP(
        tensor=ap.tensor, offset=ap.offset, ap=[ap.ap[0], [0, 2], ap.ap[1]]
    )


_BUILD_CACHE = {}


def _build(C: int):
    """Trace the single-core Bass program for capacity C (SPMD across 8)."""
    key = (C, _DT)
    if key in _BUILD_CACHE:
        return _BUILD_CACHE[key]

    dt = _mm_dt()
    nc = bass.Bass("TRN2", target_bir_lowering=False, debug=False)
    xT = nc.dram_tensor("xT", [D_IN, C], dt, kind="ExternalInput").ap()
    w1 = nc.dram_tensor("w1", [D_IN, D_HID], dt, kind="ExternalInput").ap()
    w1m = nc.dram_tensor("w1m", [D_IN], dt, kind="ExternalInput").ap()
    b1m = nc.dram_tensor("b1m", [1, 1], _F32, kind="ExternalInput").ap()
    b1 = nc.dram_tensor("b1", [D_HID], _F32, kind="ExternalInput").ap()
    gamma = nc.dram_tensor("gamma", [D_HID], _F32, kind="ExternalInput").ap()
    beta = nc.dram_tensor("beta", [D_HID], _F32, kind="ExternalInput").ap()
    w2 = nc.dram_tensor("w2", [D_HID, D_OUT], dt, kind="ExternalInput").ap()
    b2 = nc.dram_tensor("b2", [D_OUT], _F32, kind="ExternalInput").ap()
    yT = nc.dram_tensor("yT", [D_OUT, C], _F32, kind="ExternalOutput").ap()

    KC = D_IN // 128  # 2 contraction chunks for MM1
    MH = D_HID // 128  # 8 hidden chunks
    MO = D_OUT // 128  # 2 output chunks
    inv_hid = 1.0 / D_HID

    # Token tiles: TOK-wide plus one remainder (C is a multiple of 128).
    widths = [TOK] * (C // TOK)
    if C % TOK:
        widths.append(C % TOK)

    with tile.TileContext(nc) as tc, ExitStack() as ctx:
        const = ctx.enter_context(tc.tile_pool(name="const", bufs=1))
        xp = ctx.enter_context(tc.tile_pool(name="xp", bufs=3))
        hpool = ctx.enter_context(tc.tile_pool(name="hpool", bufs=2))
        tpool = ctx.enter_context(tc.tile_pool(name="tpool", bufs=3))
        spool = ctx.enter_context(tc.tile_pool(name="spool", bufs=2))
        ypool = ctx.enter_context(tc.tile_pool(name="ypool", bufs=2))
        hp_ps = ctx.enter_context(tc.tile_pool(name="hp_ps", bufs=2, space="PSUM"))
        st_ps = ctx.enter_context(tc.tile_pool(name="st_ps", bufs=1, space="PSUM"))
        rep_ps = ctx.enter_context(tc.tile_pool(name="rep_ps", bufs=1, space="PSUM"))
        y_ps = ctx.enter_context(tc.tile_pool(name="y_ps", bufs=2, space="PSUM"))

        w1_sb = const.tile([128, KC, D_HID], dt)
        nc.sync.dma_start(out=w1_sb, in_=w1.rearrange("(k p) h -> p k h", p=128))
        w2_sb = const.tile([128, MH, D_OUT], dt)
        nc.sync.dma_start(out=w2_sb, in_=w2.rearrange("(k p) o -> p k o", p=128))
        w1m_sb = const.tile([128, KC], dt)
        nc.sync.dma_start(out=w1m_sb, in_=w1m.rearrange("(k p) -> p k", p=128))
        b1m_sb = const.tile([1, 1], _F32)
        nc.sync.dma_start(out=b1m_sb, in_=b1m)
        b1_sb = const.tile([128, MH], _F32)
        nc.sync.dma_start(out=b1_sb, in_=b1.rearrange("(c p) -> p c", p=128))
        gamma_sb = const.tile([128, MH], _F32)
        nc.sync.dma_start(out=gamma_sb, in_=gamma.rearrange("(c p) -> p c", p=128))
        beta_sb = const.tile([128, MH], _F32)
        nc.sync.dma_start(out=beta_sb, in_=beta.rearrange("(c p) -> p c", p=128))
        b2_sb = const.tile([128, MO], _F32)
        nc.sync.dma_start(out=b2_sb, in_=b2.rearrange("(j p) -> p j", p=128))
        ones_col = const.tile([128, 1], dt)
        nc.vector.memset(ones_col, 1.0)
        ones_row = const.tile([1, 128], _F16)
        nc.vector.memset(ones_row, 1.0)
        negones_row = const.tile([1, 128], _F16)
        nc.vector.memset(negones_row, -1.0)
        eps_sb = const.tile([1, 1], _F32)
        nc.vector.memset(eps_sb, LN_EPS)

        t0 = 0
        for tw in widths:
            sl = slice(t0, t0 + tw)
            t0 += tw
            xt = xp.tile([128, KC, TOK], dt, tag="xt", name="xt")[:, :, :tw]
            nc.sync.dma_start(
                out=xt, in_=xT[:, sl].rearrange("(k p) t -> p k t", p=128)
            )

            h_sb = hpool.tile([128, MH, TOK], dt, tag="h", name="h")[:, :, :tw]
            h2_sb = hpool.tile([128, MH, TOK], dt, tag="h2", name="h2")[:, :, :tw]
            hn_sb = hpool.tile([128, MH, TOK], dt, tag="hn", name="hn")[:, :, :tw]

            # MM1: hT chunks; bias applied on the PSUM->SBUF copy (ACT).
            for m in range(MH):
                hp = hp_ps.tile([128, TOK], _F32, tag="hp", name="hp")[:, :tw]
                for k in range(KC):
                    nc.tensor.matmul(
                        hp,
                        lhsT=w1_sb[:, k, m * 128 : (m + 1) * 128],
                        rhs=xt[:, k, :],
                        start=(k == 0),
                        stop=(k == KC - 1),
                    )
                nc.scalar.activation(
                    out=h_sb[:, m, :], in_=hp, func=_AF.Identity,
                    bias=b1_sb[:, m : m + 1],
                )

            # Squares for E[h^2], in pairs (no per-chunk constants involved).
            for cp in range(MH // 2):
                pr = slice(2 * cp, 2 * cp + 2)
                nc.vector.tensor_mul(h2_sb[:, pr, :], h_sb[:, pr, :], h_sb[:, pr, :])

            # Per-token mean: folded into MM1 weights (w1m = W1.mean(axis=1)).
            musum = st_ps.tile([1, TOK], _F32, tag="musum", name="musum")[:, :tw]
            for k in range(KC):
                nc.tensor.matmul(
                    musum, lhsT=w1m_sb[:, k : k + 1], rhs=xt[:, k, :],
                    start=(k == 0), stop=(k == KC - 1),
                )
            # E[h^2]*HID: partition-direction sums via ones-vector matmuls.
            sqsum = st_ps.tile([1, TOK], _F32, tag="sqsum", name="sqsum")[:, :tw]
            for c in range(MH):
                nc.tensor.matmul(
                    sqsum, lhsT=ones_col, rhs=h2_sb[:, c, :],
                    start=(c == 0), stop=(c == MH - 1),
                )

            # Finalize per-token stats ([1, tw], cheap).
            mu = spool.tile([1, TOK], _F32, tag="mu", name="mu")[:, :tw]
            nc.scalar.activation(out=mu, in_=musum, func=_AF.Identity, bias=b1m_sb)
            ex2 = spool.tile([1, TOK], _F32, tag="ex2", name="ex2")[:, :tw]
            nc.scalar.mul(ex2, sqsum, inv_hid)
            mu2 = spool.tile([1, TOK], _F32, tag="mu2", name="mu2")[:, :tw]
            nc.vector.tensor_mul(mu2, mu, mu)
            var = spool.tile([1, TOK], _F32, tag="var", name="var")[:, :tw]
            nc.vector.tensor_sub(var, ex2, mu2)
            # rstd = 1/sqrt(var+eps) = exp(-0.5*ln(var+eps)); DVE reciprocal
            # is ~6.5ns/elem on one lane, ACT Ln/Exp are far cheaper here.
            lnv = spool.tile([1, TOK], _F32, tag="lnv", name="lnv")[:, :tw]
            nc.scalar.activation(out=lnv, in_=var, func=_AF.Ln, bias=eps_sb)
            rstd = spool.tile([1, TOK], _F16, tag="rstd", name="rstd")[:, :tw]
            nc.scalar.activation(out=rstd, in_=lnv, func=_AF.Exp, scale=-0.5)
            bvec = spool.tile([1, TOK], _F16, tag="bvec", name="bvec")[:, :tw]
            nc.vector.tensor_mul(bvec, mu, rstd)

            # Broadcast per-token scalars across partitions (rank-1 matmuls):
            # arep = 1 (x) rstd ; brep = (-1) (x) (mu*rstd).
            arep = rep_ps.tile([128, TOK], _F32, tag="arep", name="arep")[:, :tw]
            nc.tensor.matmul(arep, lhsT=ones_row, rhs=rstd, start=True, stop=True)
            brep = rep_ps.tile([128, TOK], _F32, tag="brep", name="brep")[:, :tw]
            nc.tensor.matmul(brep, lhsT=negones_row, rhs=bvec, start=True, stop=True)

            # Normalize (paired DVE), then per-chunk affine+ReLU on ACT:
            # hn = Relu(gamma * ((h-mu)*rstd) + beta).
            for cp in range(MH // 2):
                pr = slice(2 * cp, 2 * cp + 2)
                t1 = tpool.tile([128, 2, TOK], _F32, tag="t1", name="t1")[:, :, :tw]
                nc.vector.tensor_mul(t1, h_sb[:, pr, :], _bcast2(arep))
                t2 = tpool.tile([128, 2, TOK], _F32, tag="t2", name="t2")[:, :, :tw]
                nc.vector.tensor_add(t2, t1, _bcast2(brep))
                for i in range(2):
                    c = 2 * cp + i
                    nc.scalar.activation(
                        out=hn_sb[:, c, :], in_=t2[:, i, :], func=_AF.Relu,
                        bias=beta_sb[:, c : c + 1], scale=gamma_sb[:, c : c + 1],
                    )

            # MM2: yT chunks.
            y_sb = ypool.tile([128, MO, TOK], _F32, tag="y", name="y")[:, :, :tw]
            for j in range(MO):
                yp = y_ps.tile([128, TOK], _F32, tag="yp", name="yp")[:, :tw]
                for k in range(MH):
                    nc.tensor.matmul(
                        yp,
                        lhsT=w2_sb[:, k, j * 128 : (j + 1) * 128],
                        rhs=hn_sb[:, k, :],
                        start=(k == 0),
                        stop=(k == MH - 1),
                    )
                nc.scalar.activation(
                    out=y_sb[:, j, :], in_=yp, func=_AF.Identity,
                    bias=b2_sb[:, j : j + 1],
                )
            nc.sync.dma_start(
                out=yT[:, sl].rearrange("(j p) t -> p j t", p=128), in_=y_sb
            )

    _split_sync_waits(nc, max_waits=1)
    _BUILD_CACHE[key] = nc
    return nc


def _prepare(inputs):
    """Host-side dispatch: sort tokens by expert, pad, transpose."""
    x = np.asarray(inputs["x"], dtype=np.float32)
    dom = np.asarray(inputs["domain_types"]).astype(np.int64)
    W1 = np.asarray(inputs["W1"], dtype=np.float32)
    b1 = np.asarray(inputs["b1"], dtype=np.float32)
    gamma = np.asarray(inputs["gamma"], dtype=np.float32)
    beta = np.asarray(inputs["beta"], dtype=np.float32)
    W2 = np.asarray(inputs["W2"], dtype=np.float32)
    b2 = np.asarray(inputs["b2"], dtype=np.float32)

    n = x.shape[0]
    order = np.argsort(dom, kind="stable")
    counts = np.bincount(dom, minlength=N_EXPERTS)
    maxc = int(counts.max())
    C = max(128, -(-maxc // 128) * 128)

    np_dt = _np_dt()
    in_maps = []
    idx_list = []
    off = 0
    for d in range(N_EXPERTS):
        nd = int(counts[d])
        idx = order[off : off + nd]
        off += nd
        idx_list.append(idx)
        xTd = np.zeros((D_IN, C), dtype=np_dt)
        xTd[:, :nd] = x[idx].T.astype(np_dt, copy=False)
        in_maps.append(
            {
                "xT": xTd,
                "w1": W1[d].astype(np_dt, copy=False),
                "w1m": W1[d].mean(axis=1).astype(np_dt, copy=False),
                "b1m": np.full((1, 1), b1[d].mean(), dtype=np.float32),
                "b1": b1[d],
                "gamma": gamma[d],
                "beta": beta[d],
                "w2": W2[d].astype(np_dt, copy=False),
                "b2": b2[d],
            }
        )
    meta = {"n": n, "C": C, "idx_list": idx_list, "out_dtype": x.dtype}
    return in_maps, meta


def _finish(results, meta):
    out = np.zeros((meta["n"], D_OUT), dtype=meta["out_dtype"])
    for d in range(N_EXPERTS):
        idx = meta["idx_list"][d]
        if len(idx):
            out[idx] = results[d]["yT"][:, : len(idx)].T
    return out


def kernel(**inputs) -> np.ndarray:
    in_maps, meta = _prepare(inputs)
    nc = _build(meta["C"])
    res = run_bass_kernel_spmd(nc, in_maps, core_ids=list(range(N_CORES)))
    return _finish(res.results, meta)
